# revision 52
# baseline (speedup 1.0000x reference)
"""Multi-head attention TRN2 kernel, head-parallel over 8 NeuronCores.

Reference computation (fp32):
    q,k,v = x@Wq, x@Wk, x@Wv          # [B,S,16*64]
    attn  = softmax(q k^T / 8)         # per head
    out   = (attn @ v) @ Wo            # [B,S,1024]

Sharding: tensor-parallel over heads. Core c owns heads (2c, 2c+1):
Wq/Wk/Wv columns [128c:128c+128], Wo rows [128c:128c+128]. Each core
produces a full-shape partial output; the host sums the 8 partials.

Device pipeline (per batch, per head, per 1024-query span):
  1. Projections (bf16): Q^T/K^T transposed [dims, S]; V natural
     [keys, dims] with per-head ones columns (softmax denominators ride
     the AV matmul for free).
  2. Scores via fp8 DoubleRow at half the bf16 cycle cost: Q and K are
     split into e4m3 hi+lo parts. K hi/lo sit stacked on the 128
     partitions (hi on the head's own 64 lanes, lo DMA-shifted to the
     other 64); the stationary AP replicates them across the DoubleRow
     pair dim with a stride-0 broadcast. Q planes (hi, lo) form the
     moving pair, replicated to both lane halves. One DoubleRow matmul
     computes the exact (Khi+Klo)^T (Qhi+Qlo) product.
  3. exp on ACT in [128, 1024] chunks (2 PSUM banks) — the engine that
     bounds the kernel at ~0.83 ns/element.
  4. AV flipped: lhsT = exp-scores [keys, 128 queries], rhs = V[keys, 65]
     -> psum [128 queries, 65]; col 64 is the denominator. Full PE-array
     utilization (the [dims, S] orientation idles half the partitions).
  5. Normalization fused into evacuation: DVE reciprocal + stride-0
     broadcast multiply.
  6. PE transpose (identity matmul) back to [dims, queries]; bf16
     out-proj; partial output DMA'd per 128-query chunk.

Emission is software-pipelined around the in-order engines: score pairs
(which feed ACT) are emitted back-to-back, and all other PE work — AV,
transpose/out-proj, the NEXT batch's projections — is queued as small
filler chunks drained between score pairs, sized so the ACT exp stream
never starves. The final tail splits evacuations across DVE and ACT.
"""

from collections import deque
from contextlib import ExitStack

import numpy as np

HEADS = 16
DH = 64
D = 1024
B = 4
S = 2048
N_CORES = 8
HPC = HEADS // N_CORES  # heads per core = 2


def build_attention_kernel(nc, b=B, s=S):
    import concourse.bass as bass
    import concourse.tile as tile
    from concourse import mybir

    bf16 = mybir.dt.bfloat16
    f8 = mybir.dt.float8e4
    f32 = mybir.dt.float32
    ts = bass.ts
    DRow = mybir.MatmulPerfMode.DoubleRow
    Exp = mybir.ActivationFunctionType.Exp
    Copy = mybir.ActivationFunctionType.Copy
    Mult = mybir.AluOpType.mult

    DC = D // 128        # contraction chunks over model dim (8)
    IC = s // 512        # projection column chunks (4)
    JC = s // 128        # key chunks (16)
    NSP = s // 1024      # 1024-query spans (2)
    OC = D // 512        # out-proj column chunks (2)
    scale = DH ** -0.5

    xT_d = nc.dram_tensor("xT", [D, b * s], bf16, kind="ExternalInput").ap()
    wq_d = nc.dram_tensor("wq", [128, DC, 128], bf16, kind="ExternalInput").ap()
    wk_d = nc.dram_tensor("wk", [128, DC, 128], bf16, kind="ExternalInput").ap()
    wv_d = nc.dram_tensor("wv", [128, DC, 128], bf16, kind="ExternalInput").ap()
    wo_d = nc.dram_tensor("wo", [128, D], bf16, kind="ExternalInput").ap()
    id_d = nc.dram_tensor("ident", [128, 128], bf16, kind="ExternalInput").ap()
    out_d = nc.dram_tensor("out_p", [b * s, D], bf16, kind="ExternalOutput").ap()

    with tile.TileContext(nc) as tc, ExitStack() as ctx:
        wpool = ctx.enter_context(tc.tile_pool(name="weights", bufs=1))
        xpool = ctx.enter_context(tc.tile_pool(name="x", bufs=2))
        qkpool = ctx.enter_context(tc.tile_pool(name="qk8", bufs=2))
        stpool = ctx.enter_context(tc.tile_pool(name="stage", bufs=2))
        vpool = ctx.enter_context(tc.tile_pool(name="v", bufs=2))
        epool = ctx.enter_context(tc.tile_pool(name="exp", bufs=2))
        aopool = ctx.enter_context(tc.tile_pool(name="avout", bufs=2))
        otpool = ctx.enter_context(tc.tile_pool(name="ott", bufs=2))
        obpool = ctx.enter_context(tc.tile_pool(name="ob", bufs=3))
        rcpool = ctx.enter_context(tc.tile_pool(name="rc", bufs=4))
        # PSUM: 8 banks = scores 2x[128,1024] (4) + AV 2x[128,4,65] (2)
        #       + shared mm pool 2x[128,512] (2)
        ps_s = ctx.enter_context(tc.tile_pool(name="pss", bufs=2, space="PSUM"))
        ps_av = ctx.enter_context(tc.tile_pool(name="psav", bufs=2, space="PSUM"))
        ps_mm = ctx.enter_context(tc.tile_pool(name="psmm", bufs=2, space="PSUM"))

        # --- persistent weights (wq/wk first: the PE warmup and the
        # critical lead-in projections depend on them) ---
        wq_sb = wpool.tile([128, DC, 128], bf16, tag="wq")
        wk_sb = wpool.tile([128, DC, 128], bf16, tag="wk")
        wv_sb = wpool.tile([128, DC, 128], bf16, tag="wv")
        wo_sb = wpool.tile([128, D], bf16, tag="wo")
        id_sb = wpool.tile([128, 128], bf16, tag="id")
        # bf16 Q/K staging for the FIRST unit's scores: skips the fp8
        # evac/shift chain on the critical lead-in path (the fp8 planes
        # are still built, as filler, for all later units).
        qtb = wpool.tile([128, 1024], bf16, tag="qtb")
        ktb = wpool.tile([128, s], bf16, tag="ktb")
        nc.sync.dma_start(wq_sb[:], wq_d[:])
        nc.sync.dma_start(wk_sb[:], wk_d[:])

        xb_t = {}
        qk_t = {}
        v_t = {}
        ao_t = {}
        ot_t = {}

        # ---------- emission thunks ----------
        def xb_load(bi, ics=None):
            """x lives in one tile per 512-column chunk: consumers only
            couple to the chunk they read (dependency tracking is
            tile-granular), and projections start after chunk 0 lands.
            DMA remaps dram rows (dc,p) -> partitions."""
            if bi not in xb_t:
                xb_t[bi] = [
                    xpool.tile([128, DC, 512], bf16, tag=f"xb{ic}",
                               name=f"xb{ic}")
                    for ic in range(IC)
                ]
            for ic in ics if ics is not None else range(IC):
                src = xT_d[:, bi * s + ic * 512 : bi * s + (ic + 1) * 512]
                nc.sync.dma_start(
                    xb_t[bi][ic][:],
                    src.rearrange("(a p) c -> p a c", p=128),
                )

        def ensure_qk(bi):
            if bi not in qk_t:
                KTs = [
                    qkpool.tile([128, s], f8, tag=f"kts{h}", name=f"kts{h}")
                    for h in range(HPC)
                ]
                qk_t[bi] = ({}, KTs)
            return qk_t[bi]

        def q_tile(bi, h, sp):
            # Q fp8 pair tiles, created on first write (batch 0 span 0
            # never builds one: it uses the bf16 staging path)
            QTs, _ = ensure_qk(bi)
            if (h, sp) not in QTs:
                QTs[(h, sp)] = qkpool.tile(
                    [128, 2, 1024], f8, tag=f"qts{h}s{sp}", name=f"qts{h}s{sp}"
                )
            return QTs[(h, sp)]

        def projq_ic(bi, ic, act_hi=False):
            xb = xb_t[bi][ic]
            psq = ps_mm.tile([128, 512], f32, tag="psmm", name="psq")
            for dc in range(DC):
                nc.tensor.matmul(
                    psq[:], lhsT=wq_sb[:, dc, :], rhs=xb[:, dc, :],
                    start=(dc == 0), stop=(dc == DC - 1),
                )
            if bi == 0 and ic < 2:
                # span-0 queries of batch 0 are consumed via the bf16
                # staging only -- no fp8 planes needed
                nc.scalar.activation(qtb[:, ts(ic, 512)], psq[:], Copy)
                return
            icc = ic % 2
            for h in range(HPC):
                A = h * 64          # this head's projection lanes
                O = 64 - A          # the other lane half
                qt = q_tile(bi, h, ic // 2)
                with nc.allow_low_precision(reason="fp8 hi/lo split"):
                    if act_hi:  # lead-in only: offload to the idle ACT
                        nc.scalar.activation(
                            qt[A : A + 64, 0, ts(icc, 512)], psq[A : A + 64, :],
                            Copy,
                        )
                    else:
                        nc.vector.tensor_copy(
                            qt[A : A + 64, 0, ts(icc, 512)], psq[A : A + 64, :]
                        )
                    nc.vector.tensor_sub(
                        qt[A : A + 64, 1, ts(icc, 512)],
                        psq[A : A + 64, :],
                        qt[A : A + 64, 0, ts(icc, 512)],
                    )
                nc.sync.dma_start(
                    qt[O : O + 64, :, ts(icc, 512)],
                    qt[A : A + 64, :, ts(icc, 512)],
                )

        def projk_ic(bi, ic, act_hi=False):
            _, KTs = ensure_qk(bi)
            xb = xb_t[bi][ic]
            psk = ps_mm.tile([128, 512], f32, tag="psmm", name="psk")
            for dc in range(DC):
                nc.tensor.matmul(
                    psk[:], lhsT=wk_sb[:, dc, :], rhs=xb[:, dc, :],
                    start=(dc == 0), stop=(dc == DC - 1),
                )
            if bi == 0:
                if act_hi:
                    nc.scalar.activation(ktb[:, ts(ic, 512)], psk[:], Copy)
                else:
                    nc.vector.tensor_copy(ktb[:, ts(ic, 512)], psk[:])
            for h in range(HPC):
                A = h * 64
                O = 64 - A
                kt = KTs[h]
                st = stpool.tile([128, 512], f8, tag="st", name="st")
                with nc.allow_low_precision(reason="fp8 hi/lo split"):
                    if act_hi:
                        nc.scalar.activation(
                            kt[A : A + 64, ts(ic, 512)], psk[A : A + 64, :], Copy
                        )
                    else:
                        nc.vector.tensor_copy(
                            kt[A : A + 64, ts(ic, 512)], psk[A : A + 64, :]
                        )
                    nc.vector.tensor_sub(
                        st[A : A + 64, :],
                        psk[A : A + 64, :],
                        kt[A : A + 64, ts(ic, 512)],
                    )
                nc.sync.dma_start(kt[O : O + 64, ts(ic, 512)], st[A : A + 64, :])

        def projv_g(bi, g):
            xb = xb_t[bi][g]
            if bi not in v_t:
                V = vpool.tile([128, JC, 130], bf16, tag="v", name="v")
                v_t[bi] = V
                nc.vector.memset(V[:, :, 64:65], 1.0)
                nc.vector.memset(V[:, :, 129:130], 1.0)
            V = v_t[bi]
            psv = ps_mm.tile([128, 512], f32, tag="psmm", name="psv")
            pv3 = psv[:].rearrange("p (a c) -> p a c", a=4)
            for k4 in range(4):
                sc = g * 4 + k4
                for dc in range(DC):
                    nc.tensor.matmul(
                        pv3[:, k4, :],
                        lhsT=xb[:, dc, ts(k4, 128)], rhs=wv_sb[:, dc, :],
                        start=(dc == 0), stop=(dc == DC - 1),
                    )
            nc.vector.tensor_copy(V[:, g * 4 : g * 4 + 4, 0:64], pv3[:, :, 0:64])
            nc.vector.tensor_copy(V[:, g * 4 : g * 4 + 4, 65:129], pv3[:, :, 64:128])

        av_ps = {}

        def av_q4(bi, h, sp, E, qg, q4, jc_lo=0, jc_hi=None):
            """One query-chunk of the flipped AV; on the 4th, normalize
            and evacuate the group. jc_lo/jc_hi allow splitting the key
            accumulation so most of the last unit's AV can interleave
            with its own score/exp stream."""
            jc_hi = JC if jc_hi is None else jc_hi
            V = v_t[bi]
            if h == 0 and qg == 0 and q4 == 0 and jc_lo == 0:
                ao_t[(bi, sp)] = aopool.tile(
                    [128, 8, 128], bf16, tag="ao", name="ao"
                )
            if q4 == 0 and jc_lo == 0:
                av_ps[(h, sp, qg)] = ps_av.tile(
                    [128, 4, 65], f32, tag="psav", name="pa"
                )
            pa = av_ps[(h, sp, qg)]
            qoff = (qg * 4 + q4) * 128
            partial = jc_lo != 0 or jc_hi != JC
            for jc in range(jc_lo, jc_hi):
                nc.tensor.matmul(
                    pa[:, q4, :],
                    lhsT=E[:, jc, qoff : qoff + 128],
                    rhs=V[:, jc, h * 65 : h * 65 + 65],
                    start=(jc == 0), stop=(jc == JC - 1),
                    skip_group_check=partial,
                )
            if q4 == 3 and jc_hi == JC:
                ao = ao_t[(bi, sp)]
                rc = rcpool.tile([128, 4], f32, tag="rc", name="rc")
                nc.vector.reciprocal(rc[:], pa[:, :, 64])
                rcb = rc[:].unsqueeze(2).broadcast_to([128, 4, 64])
                nc.vector.tensor_tensor(
                    ao[:, qg * 4 : qg * 4 + 4, h * 64 : h * 64 + 64],
                    pa[:, :, 0:64],
                    rcb,
                    Mult,
                )

        def tail_tr(bi, sp, q4, act_assist=False):
            """Transpose one 128-query chunk back to [dims, q]."""
            if q4 == 0:
                ot_t[(bi, sp)] = otpool.tile([128, 1024], bf16, tag="ott",
                                             name="ott")
            OTT = ot_t[(bi, sp)]
            ao = ao_t[(bi, sp)]
            ptile = ps_mm.tile([128, 512], f32, tag="psmm", name="ptr")
            pt = ptile[:].bitcast(bf16)[:, 0:128]
            nc.tensor.transpose(pt, ao[:, q4, :], id_sb[:])
            if act_assist:
                nc.scalar.activation(OTT[:, ts(q4, 128)], pt, Copy)
            else:
                nc.vector.tensor_copy(OTT[:, ts(q4, 128)], pt)

        def tail_op(bi, sp, q4, act_assist=False, wide_pool=False):
            """Out-projection of one 128-query chunk + output DMA. The
            final tail borrows the (then idle) 4-bank scores pool so four
            matmuls can be in flight while DVE and ACT evacuate."""
            OTT = ot_t[(bi, sp)]
            ob = obpool.tile([128, D], bf16, tag="ob", name="ob")
            if wide_pool:
                po2 = ps_s.tile([128, 1024], f32, tag="pss", name="pss")
                for oc in range(OC):
                    nc.tensor.matmul(
                        po2[:, ts(oc, 512)],
                        lhsT=OTT[:, ts(q4, 128)], rhs=wo_sb[:, ts(oc, 512)],
                        start=True, stop=True,
                    )
                row0 = bi * s + sp * 1024 + q4 * 128
                nc.vector.tensor_copy(ob[:, 0:512], po2[:, 0:512])
                nc.sync.dma_start(out_d[row0 : row0 + 128, 0:512], ob[:, 0:512])
                nc.scalar.activation(ob[:, 512:1024], po2[:, 512:1024], Copy)
                nc.sync.dma_start(
                    out_d[row0 : row0 + 128, 512:1024], ob[:, 512:1024]
                )
                if q4 == 7:
                    ao_t.pop((bi, sp))
                    ot_t.pop((bi, sp))
                return
            else:
                for oc in range(OC):
                    po = ps_mm.tile([128, 512], f32, tag="psmm", name="po")
                    nc.tensor.matmul(
                        po[:], lhsT=OTT[:, ts(q4, 128)], rhs=wo_sb[:, ts(oc, 512)],
                        start=True, stop=True,
                    )
                    if act_assist and oc == 1:
                        nc.scalar.activation(ob[:, ts(oc, 512)], po[:], Copy)
                    else:
                        nc.vector.tensor_copy(ob[:, ts(oc, 512)], po[:])
            row0 = bi * s + sp * 1024 + q4 * 128
            nc.sync.dma_start(out_d[row0 : row0 + 128, :], ob[:])
            if q4 == 7:
                ao_t.pop((bi, sp))
                ot_t.pop((bi, sp))

        def tail_q4(bi, sp, q4, act_assist=False):
            tail_tr(bi, sp, q4, act_assist)
            tail_op(bi, sp, q4, act_assist)

        # ---------- filler queue ----------
        filler = deque()

        def enq(est_ns, th):
            filler.append((est_ns, th))

        def drain(budget):
            while filler and budget > 0:
                est, th = filler.popleft()
                th()
                budget -= est

        def drain_all():
            while filler:
                filler.popleft()[1]()

        def scores_exp(bi, h, sp, split_first=0, budget=830, bf16_qk=False,
                       late=None):
            """Scores + 1024-wide exp; drains filler between score pairs
            to keep the in-order PE stream dense. split_first emits
            512-wide exps for the first jc's (lead-in). bf16_qk uses the
            bf16 staging tiles (first unit: skips the fp8 chain latency).
            `late` thunks are drained only at jc>=12 (last unit: partial
            AV whose reads must follow the exp emissions they consume)."""
            _, KTs = ensure_qk(bi)
            E = epool.tile([128, JC, 1024], bf16, tag="e", name="e")
            for jc in range(JC):
                pss = ps_s.tile([128, 1024], f32, tag="pss", name="pss")
                if not bf16_qk:
                    lhsT = (
                        KTs[h][:, ts(jc, 128)]
                        .unsqueeze(1).broadcast_to([128, 2, 128])
                    )
                for half in range(2):
                    q0 = sp * 1024 + half * 512
                    if bf16_qk:
                        nc.tensor.matmul(
                            pss[:, half * 512 : half * 512 + 512],
                            lhsT=ktb[h * 64 : h * 64 + 64, ts(jc, 128)],
                            rhs=qtb[h * 64 : h * 64 + 64, half * 512
                                    : half * 512 + 512],
                            start=True, stop=True,
                        )
                    else:
                        nc.tensor.matmul(
                            pss[:, half * 512 : half * 512 + 512],
                            lhsT=lhsT,
                            rhs=q_tile(bi, h, sp)[
                                :, :, half * 512 : half * 512 + 512],
                            start=True, stop=True, perf_mode=DRow,
                        )
                    if jc < split_first:
                        nc.scalar.activation(
                            E[:, jc, half * 512 : half * 512 + 512],
                            pss[:, half * 512 : half * 512 + 512],
                            Exp, scale=scale,
                        )
                if jc >= split_first:
                    nc.scalar.activation(E[:, jc, :], pss[:], Exp, scale=scale)
                if late is not None and jc >= 12:
                    if jc == 12:
                        # the late-AV psum allocation must not precede the
                        # still-queued frees of the previous unit's AV
                        drain_all()
                    late(jc, E)
                if jc >= 3:
                    # first pairs run un-interleaved to build ACT backlog
                    drain(budget)
            return E

        def enq_av(bi, h, sp, E):
            for qg in range(2):
                for q4 in range(4):
                    enq(470, (lambda h_=h, sp_=sp, qg_=qg, q4_=q4, E_=E:
                              av_q4(bi, h_, sp_, E_, qg_, q4_)))

        def enq_tail(bi, sp):
            for q4 in range(8):
                enq(490, (lambda bi_=bi, sp_=sp, q4_=q4:
                          tail_q4(bi_, sp_, q4_)))

        # ---------- schedule ----------
        UNITS = [(0, 0), (1, 0), (0, 1), (1, 1)]
        # Lead-in: the first score pairs need only xb ic0/ic1, K-ic0 and
        # Q span 0 (per-ic x tiles decouple the rest).
        xb_load(0, [0, 1])
        # PE warmup: throwaway matmuls on the already-landed wq tile so
        # the p-state ramp completes before the real projections.
        wu = ps_mm.tile([128, 512], f32, tag="psmm", name="wu")
        for _ in range(16):
            nc.tensor.matmul(
                wu[:, 0:128], lhsT=wq_sb[:, 0, :], rhs=wq_sb[:, 0, :],
                start=True, stop=True,
            )
        projq_ic(0, 0, act_hi=True)
        projk_ic(0, 0, act_hi=True)
        projq_ic(0, 1, act_hi=True)
        nc.sync.dma_start(wv_sb[:], wv_d[:])
        nc.sync.dma_start(wo_sb[:], wo_d[:])
        nc.sync.dma_start(id_sb[:], id_d[:])
        enq(1800, lambda: projk_ic(0, 1))
        enq(100, lambda: xb_load(0, [2, 3]))
        for ic in (2, 3):
            enq(1800, (lambda ic_=ic: projk_ic(0, ic_)))
        for g in range(4):
            enq(1800, (lambda g_=g: projv_g(0, g_)))
        for ic in (2, 3):
            enq(1800, (lambda ic_=ic: projq_ic(0, ic_)))

        prev = None
        for bi in range(b):
            for ui, (h, sp) in enumerate(UNITS):
                E = scores_exp(
                    bi, h, sp,
                    split_first=(6 if (bi, ui) == (0, 0) else 0),
                    budget=(650 if bi == 0 and ui < 2 else 830),
                    bf16_qk=(bi == 0 and ui < 2),
                )
                if prev is not None:
                    enq_av(*prev)
                    if prev[1] == 1:  # both heads of that span done
                        enq_tail(prev[0], prev[2])
                if bi + 1 < b:
                    if ui == 0:
                        xb_load(bi + 1)
                    elif ui == 1:
                        for ic in range(IC):
                            enq(1800, (lambda bi_=bi + 1, ic_=ic: projk_ic(bi_, ic_)))
                            enq(1800, (lambda bi_=bi + 1, ic_=ic: projq_ic(bi_, ic_)))
                    elif ui == 3:
                        for g in range(4):
                            enq(1800, (lambda bi_=bi + 1, g_=g: projv_g(bi_, g_)))
                prev = (bi, h, sp, E)

        # Final tail: no more exps -- drain everything, then run the last
        # unit's AV and its transpose/out-proj as two dense pipelined
        # phases, splitting evacuations across DVE and ACT.
        drain_all()
        bi, h, sp, E = prev
        for qg in range(2):
            for q4 in range(4):
                av_q4(bi, h, sp, E, qg, q4)
        for q4 in range(8):
            tail_tr(bi, sp, q4, act_assist=(q4 % 2 == 1))
        for q4 in range(8):
            tail_op(bi, sp, q4, act_assist=True)
    return nc


_NC_CACHE = {}


def _make_nc(b=B, s=S, compile=True):
    from concourse import bacc

    key = (b, s, compile)
    if key in _NC_CACHE:
        return _NC_CACHE[key]
    nc = bacc.Bacc("TRN2", target_bir_lowering=False, debug=False, num_devices=N_CORES)
    build_attention_kernel(nc, b=b, s=s)
    if compile:
        nc.compile()
    _NC_CACHE[key] = nc
    return nc


def _wslice(W, sl):
    """[1024, 128] weight slice -> [128, DC, 128] (partition-major chunks)."""
    import ml_dtypes

    w = np.asarray(W, np.float32)[:, sl]
    return np.ascontiguousarray(
        w.reshape(D // 128, 128, 128).transpose(1, 0, 2)
    ).astype(ml_dtypes.bfloat16)


def kernel(x, Wq, Wk, Wv, Wo, _trace=False):
    import ml_dtypes
    from concourse import bass_utils

    bf16 = ml_dtypes.bfloat16
    x = np.asarray(x, dtype=np.float32)
    b, s, d = x.shape
    flat = np.ascontiguousarray(x.reshape(b * s, d))
    xT = np.ascontiguousarray(flat.T).astype(bf16)
    ident = np.eye(128, dtype=np.float32).astype(bf16)

    nc = _make_nc(b=b, s=s)

    in_maps = []
    for c in range(N_CORES):
        sl = slice(c * 128, (c + 1) * 128)
        in_maps.append(
            {
                "xT": xT,
                "wq": _wslice(Wq, sl),
                "wk": _wslice(Wk, sl),
                "wv": _wslice(Wv, sl),
                "wo": np.ascontiguousarray(np.asarray(Wo, np.float32)[sl, :]).astype(bf16),
                "ident": ident,
            }
        )

    res = bass_utils.run_bass_kernel_spmd(
        nc, in_maps, core_ids=list(range(N_CORES)), trace=_trace
    )
    acc = np.zeros((b * s, d), np.float32)
    for r in res.results:
        acc += np.asarray(r["out_p"], np.float32)
    out = acc.reshape(b, s, d)
    if _trace:
        kernel._last_results = res
    return out


# revision 56
# speedup vs baseline: 1.0046x; 1.0046x over previous
"""Multi-head attention TRN2 kernel, head-parallel over 8 NeuronCores.

Reference computation (fp32):
    q,k,v = x@Wq, x@Wk, x@Wv          # [B,S,16*64]
    attn  = softmax(q k^T / 8)         # per head
    out   = (attn @ v) @ Wo            # [B,S,1024]

Sharding: tensor-parallel over heads. Core c owns heads (2c, 2c+1):
Wq/Wk/Wv columns [128c:128c+128], Wo rows [128c:128c+128]. Each core
produces a full-shape partial output; the host sums the 8 partials.

Device pipeline (per batch, per head, per 1024-query span):
  1. Projections (bf16): Q^T/K^T transposed [dims, S]; V natural
     [keys, dims] with per-head ones columns (softmax denominators ride
     the AV matmul for free).
  2. Scores via fp8 DoubleRow at half the bf16 cycle cost: Q and K are
     split into e4m3 hi+lo parts. K hi/lo sit stacked on the 128
     partitions (hi on the head's own 64 lanes, lo DMA-shifted to the
     other 64); the stationary AP replicates them across the DoubleRow
     pair dim with a stride-0 broadcast. Q planes (hi, lo) form the
     moving pair, replicated to both lane halves. One DoubleRow matmul
     computes the exact (Khi+Klo)^T (Qhi+Qlo) product.
  3. exp on ACT in [128, 1024] chunks (2 PSUM banks) — the engine that
     bounds the kernel at ~0.83 ns/element.
  4. AV flipped: lhsT = exp-scores [keys, 128 queries], rhs = V[keys, 65]
     -> psum [128 queries, 65]; col 64 is the denominator. Full PE-array
     utilization (the [dims, S] orientation idles half the partitions).
  5. Normalization fused into evacuation: DVE reciprocal + stride-0
     broadcast multiply.
  6. PE transpose (identity matmul) back to [dims, queries]; bf16
     out-proj; partial output DMA'd per 128-query chunk.

Emission is software-pipelined around the in-order engines: score pairs
(which feed ACT) are emitted back-to-back, and all other PE work — AV,
transpose/out-proj, the NEXT batch's projections — is queued as small
filler chunks drained between score pairs, sized so the ACT exp stream
never starves. The final tail splits evacuations across DVE and ACT.
"""

from collections import deque
from contextlib import ExitStack

import numpy as np

HEADS = 16
DH = 64
D = 1024
B = 4
S = 2048
N_CORES = 8
HPC = HEADS // N_CORES  # heads per core = 2


def build_attention_kernel(nc, b=B, s=S):
    import concourse.bass as bass
    import concourse.tile as tile
    from concourse import mybir

    bf16 = mybir.dt.bfloat16
    f8 = mybir.dt.float8e4
    f32 = mybir.dt.float32
    ts = bass.ts
    DRow = mybir.MatmulPerfMode.DoubleRow
    Exp = mybir.ActivationFunctionType.Exp
    Copy = mybir.ActivationFunctionType.Copy
    Mult = mybir.AluOpType.mult

    DC = D // 128        # contraction chunks over model dim (8)
    IC = s // 512        # projection column chunks (4)
    JC = s // 128        # key chunks (16)
    NSP = s // 1024      # 1024-query spans (2)
    OC = D // 512        # out-proj column chunks (2)
    scale = DH ** -0.5

    xT_d = nc.dram_tensor("xT", [D, b * s], bf16, kind="ExternalInput").ap()
    wq_d = nc.dram_tensor("wq", [128, DC, 128], bf16, kind="ExternalInput").ap()
    wk_d = nc.dram_tensor("wk", [128, DC, 128], bf16, kind="ExternalInput").ap()
    wv_d = nc.dram_tensor("wv", [128, DC, 128], bf16, kind="ExternalInput").ap()
    wo_d = nc.dram_tensor("wo", [128, D], bf16, kind="ExternalInput").ap()
    id_d = nc.dram_tensor("ident", [128, 128], bf16, kind="ExternalInput").ap()
    out_d = nc.dram_tensor("out_p", [b * s, D], bf16, kind="ExternalOutput").ap()

    with tile.TileContext(nc) as tc, ExitStack() as ctx:
        wpool = ctx.enter_context(tc.tile_pool(name="weights", bufs=1))
        xpool = ctx.enter_context(tc.tile_pool(name="x", bufs=2))
        qkpool = ctx.enter_context(tc.tile_pool(name="qk8", bufs=2))
        stpool = ctx.enter_context(tc.tile_pool(name="stage", bufs=2))
        vpool = ctx.enter_context(tc.tile_pool(name="v", bufs=2))
        epool = ctx.enter_context(tc.tile_pool(name="exp", bufs=2))
        aopool = ctx.enter_context(tc.tile_pool(name="avout", bufs=2))
        otpool = ctx.enter_context(tc.tile_pool(name="ott", bufs=2))
        obpool = ctx.enter_context(tc.tile_pool(name="ob", bufs=3))
        rcpool = ctx.enter_context(tc.tile_pool(name="rc", bufs=4))
        # PSUM: 8 banks = scores 2x[128,1024] (4) + AV 2x[128,4,65] (2)
        #       + shared mm pool 2x[128,512] (2)
        ps_s = ctx.enter_context(tc.tile_pool(name="pss", bufs=2, space="PSUM"))
        ps_av = ctx.enter_context(tc.tile_pool(name="psav", bufs=2, space="PSUM"))
        ps_mm = ctx.enter_context(tc.tile_pool(name="psmm", bufs=2, space="PSUM"))

        # --- persistent weights (wq/wk first: the PE warmup and the
        # critical lead-in projections depend on them) ---
        wq_sb = wpool.tile([128, DC, 128], bf16, tag="wq")
        wk_sb = wpool.tile([128, DC, 128], bf16, tag="wk")
        wv_sb = wpool.tile([128, DC, 128], bf16, tag="wv")
        wo_sb = wpool.tile([128, D], bf16, tag="wo")
        id_sb = wpool.tile([128, 128], bf16, tag="id")
        # bf16 Q/K staging for the FIRST unit's scores: skips the fp8
        # evac/shift chain on the critical lead-in path (the fp8 planes
        # are still built, as filler, for all later units).
        qtb = wpool.tile([128, 1024], bf16, tag="qtb")
        ktb = wpool.tile([128, s], bf16, tag="ktb")

        xb_t = {}
        qk_t = {}
        v_t = {}
        ao_t = {}
        ot_t = {}

        # ---------- emission thunks ----------
        def xb_load(bi, ics=None):
            """x lives in one tile per 512-column chunk: consumers only
            couple to the chunk they read (dependency tracking is
            tile-granular), and projections start after chunk 0 lands.
            DMA remaps dram rows (dc,p) -> partitions."""
            if bi not in xb_t:
                xb_t[bi] = [
                    xpool.tile([128, DC, 512], bf16, tag=f"xb{ic}",
                               name=f"xb{ic}")
                    for ic in range(IC)
                ]
            for ic in ics if ics is not None else range(IC):
                src = xT_d[:, bi * s + ic * 512 : bi * s + (ic + 1) * 512]
                nc.sync.dma_start(
                    xb_t[bi][ic][:],
                    src.rearrange("(a p) c -> p a c", p=128),
                )

        def ensure_qk(bi):
            if bi not in qk_t:
                KTs = [
                    qkpool.tile([128, s], f8, tag=f"kts{h}", name=f"kts{h}")
                    for h in range(HPC)
                ]
                qk_t[bi] = ({}, KTs)
            return qk_t[bi]

        def q_tile(bi, h, sp):
            # Q fp8 pair tiles, created on first write (batch 0 span 0
            # never builds one: it uses the bf16 staging path)
            QTs, _ = ensure_qk(bi)
            if (h, sp) not in QTs:
                QTs[(h, sp)] = qkpool.tile(
                    [128, 2, 1024], f8, tag=f"qts{h}s{sp}", name=f"qts{h}s{sp}"
                )
            return QTs[(h, sp)]

        def projq_ic(bi, ic, act_hi=False):
            xb = xb_t[bi][ic]
            psq = ps_mm.tile([128, 512], f32, tag="psmm", name="psq")
            for dc in range(DC):
                nc.tensor.matmul(
                    psq[:], lhsT=wq_sb[:, dc, :], rhs=xb[:, dc, :],
                    start=(dc == 0), stop=(dc == DC - 1),
                )
            if bi == 0 and ic < 2:
                # span-0 queries of batch 0 are consumed via the bf16
                # staging only -- no fp8 planes needed
                nc.scalar.activation(qtb[:, ts(ic, 512)], psq[:], Copy)
                return
            icc = ic % 2
            for h in range(HPC):
                A = h * 64          # this head's projection lanes
                O = 64 - A          # the other lane half
                qt = q_tile(bi, h, ic // 2)
                with nc.allow_low_precision(reason="fp8 hi/lo split"):
                    if act_hi:  # lead-in only: offload to the idle ACT
                        nc.scalar.activation(
                            qt[A : A + 64, 0, ts(icc, 512)], psq[A : A + 64, :],
                            Copy,
                        )
                    else:
                        nc.vector.tensor_copy(
                            qt[A : A + 64, 0, ts(icc, 512)], psq[A : A + 64, :]
                        )
                    nc.vector.tensor_sub(
                        qt[A : A + 64, 1, ts(icc, 512)],
                        psq[A : A + 64, :],
                        qt[A : A + 64, 0, ts(icc, 512)],
                    )
                nc.sync.dma_start(
                    qt[O : O + 64, :, ts(icc, 512)],
                    qt[A : A + 64, :, ts(icc, 512)],
                )

        def projk_ic(bi, ic, act_hi=False):
            _, KTs = ensure_qk(bi)
            xb = xb_t[bi][ic]
            psk = ps_mm.tile([128, 512], f32, tag="psmm", name="psk")
            for dc in range(DC):
                nc.tensor.matmul(
                    psk[:], lhsT=wk_sb[:, dc, :], rhs=xb[:, dc, :],
                    start=(dc == 0), stop=(dc == DC - 1),
                )
            if bi == 0:
                if act_hi:
                    nc.scalar.activation(ktb[:, ts(ic, 512)], psk[:], Copy)
                else:
                    nc.vector.tensor_copy(ktb[:, ts(ic, 512)], psk[:])
            for h in range(HPC):
                A = h * 64
                O = 64 - A
                kt = KTs[h]
                st = stpool.tile([128, 512], f8, tag="st", name="st")
                with nc.allow_low_precision(reason="fp8 hi/lo split"):
                    if act_hi:
                        nc.scalar.activation(
                            kt[A : A + 64, ts(ic, 512)], psk[A : A + 64, :], Copy
                        )
                    else:
                        nc.vector.tensor_copy(
                            kt[A : A + 64, ts(ic, 512)], psk[A : A + 64, :]
                        )
                    nc.vector.tensor_sub(
                        st[A : A + 64, :],
                        psk[A : A + 64, :],
                        kt[A : A + 64, ts(ic, 512)],
                    )
                nc.sync.dma_start(kt[O : O + 64, ts(ic, 512)], st[A : A + 64, :])

        def projv_g(bi, g):
            xb = xb_t[bi][g]
            if bi not in v_t:
                V = vpool.tile([128, JC, 130], bf16, tag="v", name="v")
                v_t[bi] = V
                nc.vector.memset(V[:, :, 64:65], 1.0)
                nc.vector.memset(V[:, :, 129:130], 1.0)
            V = v_t[bi]
            psv = ps_mm.tile([128, 512], f32, tag="psmm", name="psv")
            pv3 = psv[:].rearrange("p (a c) -> p a c", a=4)
            for k4 in range(4):
                sc = g * 4 + k4
                for dc in range(DC):
                    nc.tensor.matmul(
                        pv3[:, k4, :],
                        lhsT=xb[:, dc, ts(k4, 128)], rhs=wv_sb[:, dc, :],
                        start=(dc == 0), stop=(dc == DC - 1),
                    )
            nc.vector.tensor_copy(V[:, g * 4 : g * 4 + 4, 0:64], pv3[:, :, 0:64])
            nc.vector.tensor_copy(V[:, g * 4 : g * 4 + 4, 65:129], pv3[:, :, 64:128])

        av_ps = {}

        def av_q4(bi, h, sp, E, qg, q4, jc_lo=0, jc_hi=None):
            """One query-chunk of the flipped AV; on the 4th, normalize
            and evacuate the group. jc_lo/jc_hi allow splitting the key
            accumulation so most of the last unit's AV can interleave
            with its own score/exp stream."""
            jc_hi = JC if jc_hi is None else jc_hi
            V = v_t[bi]
            if h == 0 and qg == 0 and q4 == 0 and jc_lo == 0:
                ao_t[(bi, sp)] = aopool.tile(
                    [128, 8, 128], bf16, tag="ao", name="ao"
                )
            if q4 == 0 and jc_lo == 0:
                av_ps[(h, sp, qg)] = ps_av.tile(
                    [128, 4, 65], f32, tag="psav", name="pa"
                )
            pa = av_ps[(h, sp, qg)]
            qoff = (qg * 4 + q4) * 128
            partial = jc_lo != 0 or jc_hi != JC
            for jc in range(jc_lo, jc_hi):
                nc.tensor.matmul(
                    pa[:, q4, :],
                    lhsT=E[:, jc, qoff : qoff + 128],
                    rhs=V[:, jc, h * 65 : h * 65 + 65],
                    start=(jc == 0), stop=(jc == JC - 1),
                    skip_group_check=partial,
                )
            if q4 == 3 and jc_hi == JC:
                ao = ao_t[(bi, sp)]
                rc = rcpool.tile([128, 4], f32, tag="rc", name="rc")
                nc.vector.reciprocal(rc[:], pa[:, :, 64])
                rcb = rc[:].unsqueeze(2).broadcast_to([128, 4, 64])
                nc.vector.tensor_tensor(
                    ao[:, qg * 4 : qg * 4 + 4, h * 64 : h * 64 + 64],
                    pa[:, :, 0:64],
                    rcb,
                    Mult,
                )

        def tail_tr(bi, sp, q4, act_assist=False):
            """Transpose one 128-query chunk back to [dims, q]."""
            if q4 == 0:
                ot_t[(bi, sp)] = otpool.tile([128, 1024], bf16, tag="ott",
                                             name="ott")
            OTT = ot_t[(bi, sp)]
            ao = ao_t[(bi, sp)]
            ptile = ps_mm.tile([128, 512], f32, tag="psmm", name="ptr")
            pt = ptile[:].bitcast(bf16)[:, 0:128]
            nc.tensor.transpose(pt, ao[:, q4, :], id_sb[:])
            if act_assist:
                nc.scalar.activation(OTT[:, ts(q4, 128)], pt, Copy)
            else:
                nc.vector.tensor_copy(OTT[:, ts(q4, 128)], pt)

        def tail_op(bi, sp, q4, act_assist=False, wide_pool=False):
            """Out-projection of one 128-query chunk + output DMA. The
            final tail borrows the (then idle) 4-bank scores pool so four
            matmuls can be in flight while DVE and ACT evacuate."""
            OTT = ot_t[(bi, sp)]
            ob = obpool.tile([128, D], bf16, tag="ob", name="ob")
            if wide_pool:
                po2 = ps_s.tile([128, 1024], f32, tag="pss", name="pss")
                for oc in range(OC):
                    nc.tensor.matmul(
                        po2[:, ts(oc, 512)],
                        lhsT=OTT[:, ts(q4, 128)], rhs=wo_sb[:, ts(oc, 512)],
                        start=True, stop=True,
                    )
                row0 = bi * s + sp * 1024 + q4 * 128
                nc.vector.tensor_copy(ob[:, 0:512], po2[:, 0:512])
                nc.sync.dma_start(out_d[row0 : row0 + 128, 0:512], ob[:, 0:512])
                nc.scalar.activation(ob[:, 512:1024], po2[:, 512:1024], Copy)
                nc.sync.dma_start(
                    out_d[row0 : row0 + 128, 512:1024], ob[:, 512:1024]
                )
                if q4 == 7:
                    ao_t.pop((bi, sp))
                    ot_t.pop((bi, sp))
                return
            else:
                for oc in range(OC):
                    po = ps_mm.tile([128, 512], f32, tag="psmm", name="po")
                    nc.tensor.matmul(
                        po[:], lhsT=OTT[:, ts(q4, 128)], rhs=wo_sb[:, ts(oc, 512)],
                        start=True, stop=True,
                    )
                    if act_assist and oc == 1:
                        nc.scalar.activation(ob[:, ts(oc, 512)], po[:], Copy)
                    else:
                        nc.vector.tensor_copy(ob[:, ts(oc, 512)], po[:])
            row0 = bi * s + sp * 1024 + q4 * 128
            nc.sync.dma_start(out_d[row0 : row0 + 128, :], ob[:])
            if q4 == 7:
                ao_t.pop((bi, sp))
                ot_t.pop((bi, sp))

        def tail_q4(bi, sp, q4, act_assist=False):
            tail_tr(bi, sp, q4, act_assist)
            tail_op(bi, sp, q4, act_assist)

        # ---------- filler queue ----------
        filler = deque()

        def enq(est_ns, th):
            filler.append((est_ns, th))

        def drain(budget):
            while filler and budget > 0:
                est, th = filler.popleft()
                th()
                budget -= est

        def drain_all():
            while filler:
                filler.popleft()[1]()

        def scores_exp(bi, h, sp, split_first=0, budget=830, bf16_qk=False,
                       late=None):
            """Scores + 1024-wide exp; drains filler between score pairs
            to keep the in-order PE stream dense. split_first emits
            512-wide exps for the first jc's (lead-in). bf16_qk uses the
            bf16 staging tiles (first unit: skips the fp8 chain latency).
            `late` thunks are drained only at jc>=12 (last unit: partial
            AV whose reads must follow the exp emissions they consume)."""
            _, KTs = ensure_qk(bi)
            E = epool.tile([128, JC, 1024], bf16, tag="e", name="e")
            for jc in range(JC):
                pss = ps_s.tile([128, 1024], f32, tag="pss", name="pss")
                if not bf16_qk:
                    lhsT = (
                        KTs[h][:, ts(jc, 128)]
                        .unsqueeze(1).broadcast_to([128, 2, 128])
                    )
                for half in range(2):
                    q0 = sp * 1024 + half * 512
                    if bf16_qk:
                        nc.tensor.matmul(
                            pss[:, half * 512 : half * 512 + 512],
                            lhsT=ktb[h * 64 : h * 64 + 64, ts(jc, 128)],
                            rhs=qtb[h * 64 : h * 64 + 64, half * 512
                                    : half * 512 + 512],
                            start=True, stop=True,
                        )
                    else:
                        nc.tensor.matmul(
                            pss[:, half * 512 : half * 512 + 512],
                            lhsT=lhsT,
                            rhs=q_tile(bi, h, sp)[
                                :, :, half * 512 : half * 512 + 512],
                            start=True, stop=True, perf_mode=DRow,
                        )
                    if jc < split_first:
                        nc.scalar.activation(
                            E[:, jc, half * 512 : half * 512 + 512],
                            pss[:, half * 512 : half * 512 + 512],
                            Exp, scale=scale,
                        )
                if jc >= split_first:
                    nc.scalar.activation(E[:, jc, :], pss[:], Exp, scale=scale)
                if late is not None and jc >= 12:
                    if jc == 12:
                        # the late-AV psum allocation must not precede the
                        # still-queued frees of the previous unit's AV
                        drain_all()
                    late(jc, E)
                if jc >= 3:
                    # first pairs run un-interleaved to build ACT backlog
                    drain(budget)
            return E

        def enq_av(bi, h, sp, E):
            for qg in range(2):
                for q4 in range(4):
                    enq(470, (lambda h_=h, sp_=sp, qg_=qg, q4_=q4, E_=E:
                              av_q4(bi, h_, sp_, E_, qg_, q4_)))

        def enq_tail(bi, sp):
            for q4 in range(8):
                enq(490, (lambda bi_=bi, sp_=sp, q4_=q4:
                          tail_q4(bi_, sp_, q4_)))

        # ---------- schedule ----------
        UNITS = [(0, 0), (1, 0), (0, 1), (1, 1)]
        # Lead-in DMA order: the first wq chunk lands in ~100ns so the PE
        # warmup starts immediately; xb-ic0 follows (gates the first
        # projection), then the remaining weights.
        nc.sync.dma_start(wq_sb[:, 0:1, :], wq_d[:, 0:1, :])
        xb_load(0, [0])
        nc.sync.dma_start(wq_sb[:, 1:DC, :], wq_d[:, 1:DC, :])
        nc.sync.dma_start(wk_sb[:], wk_d[:])
        xb_load(0, [1])
        # PE warmup: throwaway matmuls on the first wq chunk so the
        # p-state ramp completes before the real projections.
        wu = ps_mm.tile([128, 512], f32, tag="psmm", name="wu")
        for _ in range(16):
            nc.tensor.matmul(
                wu[:, 0:128], lhsT=wq_sb[:, 0, :], rhs=wq_sb[:, 0, :],
                start=True, stop=True,
            )
        projq_ic(0, 0, act_hi=True)
        projk_ic(0, 0, act_hi=True)
        projq_ic(0, 1, act_hi=True)
        nc.sync.dma_start(wv_sb[:], wv_d[:])
        nc.sync.dma_start(wo_sb[:], wo_d[:])
        nc.sync.dma_start(id_sb[:], id_d[:])
        enq(1800, lambda: projk_ic(0, 1))
        enq(100, lambda: xb_load(0, [2, 3]))
        for ic in (2, 3):
            enq(1800, (lambda ic_=ic: projk_ic(0, ic_)))
        for g in range(4):
            enq(1800, (lambda g_=g: projv_g(0, g_)))
        for ic in (2, 3):
            enq(1800, (lambda ic_=ic: projq_ic(0, ic_)))

        prev = None
        for bi in range(b):
            for ui, (h, sp) in enumerate(UNITS):
                E = scores_exp(
                    bi, h, sp,
                    split_first=(6 if (bi, ui) == (0, 0) else 0),
                    budget=(650 if bi == 0 and ui < 2 else 830),
                    bf16_qk=(bi == 0 and ui < 2),
                )
                if prev is not None:
                    enq_av(*prev)
                    if prev[1] == 1:  # both heads of that span done
                        enq_tail(prev[0], prev[2])
                if bi + 1 < b:
                    if ui == 0:
                        xb_load(bi + 1)
                    elif ui == 1:
                        for ic in range(IC):
                            enq(1800, (lambda bi_=bi + 1, ic_=ic: projk_ic(bi_, ic_)))
                            enq(1800, (lambda bi_=bi + 1, ic_=ic: projq_ic(bi_, ic_)))
                    elif ui == 3:
                        for g in range(4):
                            enq(1800, (lambda bi_=bi + 1, g_=g: projv_g(bi_, g_)))
                prev = (bi, h, sp, E)

        # Final tail: no more exps -- drain everything, then run the last
        # unit's AV and its transpose/out-proj as two dense pipelined
        # phases, splitting evacuations across DVE and ACT.
        drain_all()
        bi, h, sp, E = prev
        for qg in range(2):
            for q4 in range(4):
                av_q4(bi, h, sp, E, qg, q4)
        for q4 in range(8):
            tail_tr(bi, sp, q4, act_assist=(q4 % 2 == 1))
        for q4 in range(8):
            tail_op(bi, sp, q4, act_assist=True)
    return nc


_NC_CACHE = {}


def _make_nc(b=B, s=S, compile=True):
    from concourse import bacc

    key = (b, s, compile)
    if key in _NC_CACHE:
        return _NC_CACHE[key]
    nc = bacc.Bacc("TRN2", target_bir_lowering=False, debug=False, num_devices=N_CORES)
    build_attention_kernel(nc, b=b, s=s)
    if compile:
        nc.compile()
    _NC_CACHE[key] = nc
    return nc


def _wslice(W, sl):
    """[1024, 128] weight slice -> [128, DC, 128] (partition-major chunks)."""
    import ml_dtypes

    w = np.asarray(W, np.float32)[:, sl]
    return np.ascontiguousarray(
        w.reshape(D // 128, 128, 128).transpose(1, 0, 2)
    ).astype(ml_dtypes.bfloat16)


def kernel(x, Wq, Wk, Wv, Wo, _trace=False):
    import ml_dtypes
    from concourse import bass_utils

    bf16 = ml_dtypes.bfloat16
    x = np.asarray(x, dtype=np.float32)
    b, s, d = x.shape
    flat = np.ascontiguousarray(x.reshape(b * s, d))
    xT = np.ascontiguousarray(flat.T).astype(bf16)
    ident = np.eye(128, dtype=np.float32).astype(bf16)

    nc = _make_nc(b=b, s=s)

    in_maps = []
    for c in range(N_CORES):
        sl = slice(c * 128, (c + 1) * 128)
        in_maps.append(
            {
                "xT": xT,
                "wq": _wslice(Wq, sl),
                "wk": _wslice(Wk, sl),
                "wv": _wslice(Wv, sl),
                "wo": np.ascontiguousarray(np.asarray(Wo, np.float32)[sl, :]).astype(bf16),
                "ident": ident,
            }
        )

    res = bass_utils.run_bass_kernel_spmd(
        nc, in_maps, core_ids=list(range(N_CORES)), trace=_trace
    )
    acc = np.zeros((b * s, d), np.float32)
    for r in res.results:
        acc += np.asarray(r["out_p"], np.float32)
    out = acc.reshape(b, s, d)
    if _trace:
        kernel._last_results = res
    return out
